# revision 48
# baseline (speedup 1.0000x reference)
"""AutoCorrelation (FFT-free) kernel for 8 Trainium2 NeuronCores.

Math: the reference computes, per (b, h, e), the circular cross-correlation
corr = irfft(rfft(q) * conj(rfft(k))), then
  mean_corr[b, l] = mean_{h,e} corr          (only this is ever used)
  global_mean[l]  = mean_b mean_corr
  topk lags       = top-7 of global_mean
  weights         = softmax(mean_corr[:, topk])
  out[b,l]        = sum_k w[b,k] * v[b, (l - lag_k) % L]

Identity used: mean_corr[b, l] = (1/HE) * sum_s <q[b,(s+l)%L,:,:], k[b,s,:,:]>.
So instead of FFTs we compute, per batch, the Gram matrix G[s,t] = sum_c
kT[c,s] qT[c,t] on the TensorEngine (fp16 inputs, fp32 PSUM accumulation) and
fold its wrapped diagonals: the fold is fused into the matmul by rotating each
s-chunk's output columns in PSUM, leaving mean_corr[l] = sum_p S[p,(l+p)%L].

That remaining per-partition circular-diagonal fold is done with a skewed
DRAM access pattern instead of a rotate-add tree: S (plus a 128-column
replica) is written to DRAM rows of pitch 1664, then read back through a
pitch-1665 view of the same buffer, which lands S[p, (l+p)%L] at [p, l].
One gpsimd partition_all_reduce then yields the folded vector in a single
instruction. Batch 0's fold runs during batch 1's Gram; after the last
matmul only the COMBINED fold (S0+S1, by linearity = mc0+mc1) is on the
critical path, and batch 1's 7 gathered weights are recovered as
2*gl_local - mc0 without ever folding S1 alone.

Sharding: batch across the 8 cores (2 per core). Only global_mean needs an
AllReduce of a [1,1536] fp32 vector. Top-7 via the DVE max/max_index
instruction (split in halves so it overlaps the fold quarters). The topk
lags become engine registers driving dynamic access-pattern offsets into a
doubled v buffer. The weighted circular gather-sum is lt-split: the PE
covers 17 of the 24 (batch, chunk, 512-col) output slices as w-scaled
identity matmuls over 6 rotating 1-bank PSUM slices (rotating tiles
per-slice, not per-pair, so evictions never stall the next matmul group),
while the DVE covers 7 slices as 4x/2x-mode scale/add tap chains written
out with no PSUM eviction. Weights use an unnormalized softmax
(|mean_corr| <= ~8 so exp(x/C) stays in fp16/fp32 range) with 1/sum folded
into the eviction scale / tap scale; batch 0's exp->broadcast->Iw chain is
emitted first so it alone gates the first output matmuls. Loads are
ordered k0/q0 (chunk 0 in column halves) -> k1/q1 -> v on one queue: the
cost model's HWDGE and DMA units are single-slot, so issue order is
transfer order, and the Gram's cc-outer loop starts ~3us in on chunk 0
alone.

fp16 is safe here: top-7 global_mean gap is 1.5e-3 while fp16-input error is
<5e-4 (validated against the fp32 FFT reference), and the output tolerance is
2e-2 vs our ~7e-4.
"""

import numpy as np

B, L, H, E = 16, 1536, 8, 64
C = H * E             # 512 channels = H*E
NCORES = 8
BLOC = B // NCORES    # batches per core
NCC = C // 128        # channel chunks of 128
TOPK = 7              # int(1 * log(1536)) == 7
NJ = L // 128         # s-chunks
NLT = L // 512        # output l-tiles
PITCH = 1664          # DRAM row pitch for the skewed diagonal fold

_cache = {}
DEBUG_BUILD = False


def _build(num_cores: int):
    import concourse.bass as bass
    import concourse.bacc as bacc
    import concourse.mybir as mybir
    import concourse.tile as tile
    from concourse import bass_isa

    f16 = mybir.dt.float16
    f32 = mybir.dt.float32
    u32 = mybir.dt.uint32
    PE = mybir.EngineType.PE
    ACT = mybir.EngineType.Activation
    DVE = mybir.EngineType.DVE

    nc = bacc.Bacc(None)
    qT = nc.dram_tensor("qT", [BLOC, C, L], f16, kind="ExternalInput")
    kT = nc.dram_tensor("kT", [BLOC, C, L], f16, kind="ExternalInput")
    vT = nc.dram_tensor("vT", [BLOC, C, L], f16, kind="ExternalInput")
    out = nc.dram_tensor("out", [BLOC, C, L], f16, kind="ExternalOutput")
    ident_d = nc.inline_tensor(np.eye(128, dtype=np.float16), "identc")
    if DEBUG_BUILD:
        dbg_mc0 = nc.dram_tensor("dbg_mc0", [1, L], f32, kind="ExternalOutput")
        dbg_gl = nc.dram_tensor("dbg_gl", [1, L], f32, kind="ExternalOutput")
        dbg_idx = nc.dram_tensor("dbg_idx", [1, 8], u32, kind="ExternalOutput")
        dbg_wq = nc.dram_tensor("dbg_wq", [1, 16], f32, kind="ExternalOutput")
        dbg_ex = nc.dram_tensor("dbg_ex", [1, 16], f32, kind="ExternalOutput")
        dbg_rs = nc.dram_tensor("dbg_rs", [1, 2], f32, kind="ExternalOutput")
        dbg_rmax = nc.dram_tensor("dbg_rmax", [1, 1], f32, kind="ExternalOutput")
        dbg_wqs = nc.dram_tensor("dbg_wqs", [1, 16], f32, kind="ExternalOutput")
        dbg_wbc = nc.dram_tensor("dbg_wbc", [2, 16], f32, kind="ExternalOutput")

    def skew_view(d):
        # pitch-(PITCH+1) view of a [129, PITCH] dram tile: element [p, l]
        # aliases row p, column p+l of the pitch-PITCH writes, i.e. the
        # circular diagonal once columns [0,128) are replicated at [L,PITCH).
        flat = d[:].rearrange("a b -> (a b)")
        return flat[0:128 * (PITCH + 1)].rearrange("(a b) -> a b", b=PITCH + 1)

    with tile.TileContext(nc) as tc:
        with (
            tc.tile_pool(name="sb", bufs=1) as sb,
            tc.tile_pool(name="sps", bufs=1, space="PSUM") as sps,
            tc.tile_pool(name="obp", bufs=3) as obp,
            tc.tile_pool(name="dram", bufs=1, space="DRAM") as dram,
        ):
            ident = sb.tile([128, 128], f16, tag="ident")
            nc.sync.dma_start(ident, ident_d[:])

            # ---- loads: k0/q0 in interleaved halves so the Gram starts
            # ~6us in; k1/q1/v behind them on the (serial) HWDGE+DMA units.
            kq = {}
            for (nm, t) in (("k", kT), ("q", qT)):
                for bi in range(BLOC):
                    kqt = sb.tile([128, NCC, L], f16, tag=f"{nm}{bi}")
                    kq[(nm, bi)] = kqt
            for (nm, h) in (("k", 0), ("q", 0), ("q", 1), ("k", 1)):
                src = (kT if nm == "k" else qT)[0].rearrange(
                    "(cc p) l -> p cc l", p=128)
                nc.sync.dma_start(
                    kq[(nm, 0)][:, 0:1, 768 * h:768 * (h + 1)],
                    src[:, 0:1, 768 * h:768 * (h + 1)])
            for (a, b) in ((1, 2), (2, 4)):
                for nm in ("k", "q"):
                    src = (kT if nm == "k" else qT)[0].rearrange(
                        "(cc p) l -> p cc l", p=128)
                    nc.sync.dma_start(kq[(nm, 0)][:, a:b, :], src[:, a:b, :])
            for nm in ("k", "q"):
                src = (kT if nm == "k" else qT)[1].rearrange(
                    "(cc p) l -> p cc l", p=128)
                nc.sync.dma_start(kq[(nm, 1)][:], src)
            # v loads go last on the same queue: the single HWDGE/DMA unit
            # serves transfers in issue order, and v isn't needed until the
            # output phase.
            vv = []
            for bi in range(BLOC):
                t = sb.tile([128, NCC, 2 * L], f16, tag=f"vv{bi}")
                nc.sync.dma_start(
                    t[:, :, 0:L], vT[bi].rearrange("(cc p) l -> p cc l", p=128))
                vv.append(t)
            # duplicate halves on the VectorEngine instead of re-reading HBM
            for bi in range(BLOC):
                nc.vector.tensor_copy(vv[bi][:, :, L:2 * L], vv[bi][:, :, 0:L])

            # prime the ACT exp table off the critical path
            dume = sb.tile([1, 1], f32, tag="dume")
            nc.scalar.activation(dume, ident[0:1, 0:1],
                                 mybir.ActivationFunctionType.Exp,
                                 bias=0.0, scale=1.0)
            # init wq sentinel so the max over all 16 columns is safe
            wq = sb.tile([1, 16], f32, tag="wq")
            nc.vector.memset(wq, -1e30)

            # ---- Gram with rotated PSUM accumulation ----
            # cc-pair outer so batch 0's Gram can start once the first-half
            # (chunks 0-1) k/q DMAs land, streaming the rest underneath.
            def gram(bi):
                S = sps.tile([128, L], f32, tag=f"S{bi}")
                for cc in range(NCC):
                    for u in range(NJ):
                        r = (L - 128 * u) % L
                        segs = []
                        t0 = 0
                        while t0 < L:
                            y0 = (t0 + r) % L
                            seg = min(512 - (y0 % 512), L - t0, L - y0)
                            segs.append((t0, y0, seg))
                            t0 += seg
                        for (ts_, ys_, seg) in sorted(segs, key=lambda s: s[1]):
                            nc.tensor.matmul(
                                S[:, ys_:ys_ + seg],
                                kq[("k", bi)][:, cc, 128 * u:128 * (u + 1)],
                                kq[("q", bi)][:, cc, ts_:ts_ + seg],
                                start=(cc == 0 and u == 0),
                                stop=(cc == NCC - 1 and u == NJ - 1),
                                skip_group_check=True,
                            )
                return S

            S0 = gram(0)

            # ---- batch-0 diagonal fold (runs during batch-1 Gram) ----
            E0 = sb.tile([128, L], f32, tag="E0")
            nc.vector.tensor_copy(E0, S0)
            D0 = dram.tile([129, PITCH], f32)
            nc.sync.dma_start(D0[0:128, 0:L], E0)
            nc.scalar.dma_start(D0[0:128, L:PITCH], E0[:, 0:128])
            R0 = sb.tile([128, L], f32, tag="R0")
            nc.sync.dma_start(R0, skew_view(D0)[:, 0:L])
            mc0 = sb.tile([128, L], f32, tag="mc0")
            nc.gpsimd.partition_all_reduce(mc0, R0, channels=128,
                                           reduce_op=bass_isa.ReduceOp.add)

            S1 = gram(1)

            # ---- combined fold: gl_local = fold(S0+S1) = mc0 + mc1 ----
            # quarter-pipelined: combine -> write -> skewed read -> PAR all
            # stream through the (serial) DMA unit in 384-column quarters.
            # fp16 round trip: S entries are ~N(0, 22^2) so fp16 keeps the
            # fold's absolute error ~0.7 vs a 209 top-7/8 global_mean gap,
            # and halves the critical-path DMA transfer time. PAR upcasts to
            # fp32 internally and glp stays fp32.
            G = sb.tile([128, L], f16, tag="G")
            QL = L // 4
            Dg = dram.tile([129, PITCH], f16)
            Rg = sb.tile([128, L], f16, tag="Rg")
            glp = sb.tile([128, L], f32, tag="glp")
            sv = skew_view(Dg)
            # E0 is S0 already evicted to SBUF: keeps this add to a single
            # PSUM operand (two PSUM reads in one op fail the BIR verifier)
            for j in range(4):
                a, b = QL * j, QL * (j + 1)
                nc.vector.tensor_tensor(G[:, a:b], E0[:, a:b], S1[:, a:b],
                                        mybir.AluOpType.add)
                nc.sync.dma_start(Dg[0:128, a:b], G[:, a:b])
                if j == 0:
                    nc.scalar.dma_start(Dg[0:128, L:PITCH], G[:, 0:128])
            # read quarter j touches row columns [QL*j, QL*j+QL+127): needs
            # write quarters j..j+1 (and the replica for the last one).
            for j in range(4):
                a, b = QL * j, QL * (j + 1)
                nc.sync.dma_start(Rg[:, a:b], sv[:, a:b])
                nc.gpsimd.partition_all_reduce(glp[:, a:b], Rg[:, a:b],
                                               channels=128,
                                               reduce_op=bass_isa.ReduceOp.add)

            # ---- cross-core AllReduce of the local global-mean ----
            if num_cores > 1:
                cc_in = dram.tile([1, L], f32)
                cc_out = dram.tile([1, L], f32)
                nc.sync.dma_start(cc_in, glp[0:1, :])
                nc.gpsimd.collective_compute(
                    "AllReduce",
                    mybir.AluOpType.add,
                    replica_groups=[list(range(num_cores))],
                    ins=[cc_in.opt()],
                    outs=[cc_out.opt()],
                )
                gm = sb.tile([1, L], f32, tag="gm")
                nc.sync.dma_start(gm, cc_out)
            else:
                gm = glp[0:1, :]

            # ---- top-7 lags (top-8 instruction, first 7 used) ----
            # halves overlap the PAR quarters; merged top-8 feeds the
            # full-width index search.
            vals = sb.tile([1, 8], f32, tag="vals")
            idxs = sb.tile([1, 8], u32, tag="idxs")
            v16 = sb.tile([1, 16], f32, tag="v16")
            nc.vector.max(v16[0:1, 0:8], gm[0:1, 0:L // 2])
            nc.vector.max(v16[0:1, 8:16], gm[0:1, L // 2:L])
            nc.vector.max(vals, v16)
            nc.vector.max_index(idxs, vals, gm)

            # ---- index registers ----
            act_eng = nc.engines[ACT]
            dve_eng = nc.engines[DVE]
            pe_eng = nc.engines[PE]
            sv_x = {}    # PE: (L - idx) + 512*lt for the dynamic matmuls
            sv_gd = []   # DVE: idx for mc0 gathers
            sv_od = []   # DVE: L - idx for the cc=3 tap windows
            sv_ga = []   # ACT: idx for mc0 taps 4-6
            sv_ga2 = []  # ACT: idx for gl gathers
            # PE index registers are emitted lazily, interleaved with the
            # first output slices, so the ~100ns/op SEQ register setup hides
            # behind matmul engine time instead of gating the first Ldweights
            pe_ro = {}

            def pe_snap_lt0(k):
                rp = pe_eng.alloc_register(f"ip{k}")
                pe_eng.reg_load(rp, idxs[0:1, k:k + 1])
                ro = pe_eng.alloc_register(f"io{k}")
                pe_eng.reg_alu(ro, L, rp, mybir.AluOpType.subtract)
                pe_eng.free_register(rp)
                pe_ro[k] = ro
                sv_x[(k, 0)] = pe_eng.snap(ro, donate=False,
                                           min_val=1, max_val=L)

            def pe_snap_lt(k, lt):
                # lt 3 = the final 128-col mini-slice at offset 512+384
                base = 512 * lt if lt < 3 else 896
                rx = pe_eng.alloc_register(f"ix{k}_{lt}")
                pe_eng.reg_alu(rx, pe_ro[k], base, mybir.AluOpType.add)
                if lt == 3:
                    pe_eng.free_register(pe_ro[k])
                sv_x[(k, lt)] = pe_eng.snap(rx, donate=True,
                                            min_val=base + 1,
                                            max_val=L + base)

            for k in range(TOPK):
                ra = act_eng.alloc_register(f"ia{k}")
                act_eng.reg_load(ra, idxs[0:1, k:k + 1])
                sv_ga.append(act_eng.snap(ra, donate=True,
                                          min_val=0, max_val=L - 1))
            dve_regs = []
            for k in range(TOPK):
                rd = dve_eng.alloc_register(f"id{k}")
                dve_eng.reg_load(rd, idxs[0:1, k:k + 1])
                dve_regs.append(rd)
                sv_gd.append(dve_eng.snap(rd, donate=False,
                                          min_val=0, max_val=L - 1))

            # ---- weights: wq[0,0:7]=mc0 taps, wq[0,8:15]=gl-mc0 taps ----
            # mc0 gathers split DVE/ACT so exp-b0 fires ~0.7us after topk;
            # the gl gathers (only needed for b1) follow on ACT afterwards.
            for k in range(4):
                nc.vector.tensor_copy(wq[0:1, k:k + 1],
                                      mc0[0:1, bass.ds(sv_gd[k], 1)])
            for k in range(4, TOPK):
                nc.scalar.copy(wq[0:1, k:k + 1],
                               mc0[0:1, bass.ds(sv_ga[k], 1)])
            for k in range(TOPK):
                rdo = dve_eng.alloc_register(f"do{k}")
                dve_eng.reg_alu(rdo, L + 1024, dve_regs[k],
                                mybir.AluOpType.subtract)
                dve_eng.free_register(dve_regs[k])
                sv_od.append(dve_eng.snap(rdo, donate=True,
                                          min_val=1025, max_val=L + 1024))
            # b0's weight pipeline runs first and alone gates the first PE
            # slices; b1 (= gl - mc0 by linearity, then exp/bcast/Iw) follows
            # in its shadow. Softmax is unnormalized: |mean_corr| <~ 8 so
            # exp(x/C) <~ e^8 fits fp16/fp32; 1/sum folds into output scaling.
            ex = sb.tile([1, 16], f32, tag="ex")
            wbc = sb.tile([128, 16], f32, tag="wbc")
            rs = sb.tile([128, 2], f32, tag="rs")
            sm = sb.tile([128, 2], f32, tag="sm")
            Iw = [[None] * TOPK for _ in range(BLOC)]
            nc.scalar.activation(ex[0:1, 0:8], wq[0:1, 0:8],
                                 mybir.ActivationFunctionType.Exp,
                                 bias=0.0, scale=1.0 / C)
            nc.gpsimd.partition_broadcast(wbc[:, 0:8], ex[0:1, 0:8],
                                          channels=128)
            for k in range(TOPK):
                ra2 = act_eng.alloc_register(f"ib{k}")
                act_eng.reg_load(ra2, idxs[0:1, k:k + 1])
                sv_ga2.append(act_eng.snap(ra2, donate=True,
                                           min_val=0, max_val=L - 1))
                nc.scalar.copy(wq[0:1, 8 + k:9 + k],
                               glp[0:1, bass.ds(sv_ga2[k], 1)])
            for k in range(TOPK):
                t = sb.tile([128, 128], f16, tag=f"iw0{k}")
                nc.vector.tensor_scalar_mul(t, ident, wbc[:, k:k + 1])
                Iw[0][k] = t
            # gl_local is the SUM over this core's two batches: mc1 = gl - mc0
            nc.vector.tensor_tensor(wq[0:1, 8:15], wq[0:1, 8:15],
                                    wq[0:1, 0:7], mybir.AluOpType.subtract)
            nc.scalar.activation(ex[0:1, 8:16], wq[0:1, 8:16],
                                 mybir.ActivationFunctionType.Exp,
                                 bias=0.0, scale=1.0 / C)
            nc.gpsimd.partition_broadcast(wbc[:, 8:16], ex[0:1, 8:16],
                                          channels=128)
            for k in range(TOPK):
                t = sb.tile([128, 128], f16, tag=f"iw1{k}")
                nc.gpsimd.tensor_scalar_mul(t, ident, wbc[:, 8 + k:9 + k])
                Iw[1][k] = t
            nc.vector.tensor_reduce(sm[:, 0:1], wbc[:, 0:TOPK],
                                    mybir.AxisListType.X, mybir.AluOpType.add)
            nc.vector.tensor_reduce(sm[:, 1:2], wbc[:, 8:8 + TOPK],
                                    mybir.AxisListType.X, mybir.AluOpType.add)
            nc.vector.reciprocal(rs, sm)
            # window offsets for the DVE slices: (L + 512*lt) - idx for
            # lt in {1,2}, off the critical path (only needed once the DVE
            # tap chains start)

            if DEBUG_BUILD:
                nc.sync.dma_start(dbg_mc0[:], mc0[0:1, :])
                nc.sync.dma_start(dbg_gl[:], glp[0:1, :])
                nc.sync.dma_start(dbg_idx[:], idxs)
                nc.sync.dma_start(dbg_wq[:], wq)
                nc.sync.dma_start(dbg_ex[:], ex)
                nc.sync.dma_start(dbg_rs[:], rs[0:1, :])
                nc.sync.dma_start(dbg_wbc[:], wbc[0:2, :])

            # ---- weighted circular gather-sum ----
            # chunks 0-2: w-scaled identity matmuls, dynamic rhs offsets,
            # PSUM accumulation; eviction applies the 1/sum normalization.
            # chunk 3: DVE scale/add chain on full-L windows (runs in the
            # shadow of the PE matmuls).
            # lt-split output: PE computes lt 0-1 of every (batch, chunk)
            # pair as w-scaled identity matmuls (6 rotating 512-col PSUM
            # slices so evictions never stall the next slice's matmuls); the
            # DVE computes every lt=2 slice as a scale/add tap chain and DMAs
            # it out directly, so both engines finish together and the kernel
            # ends on an eviction-free DMA.
            psA = sps.tile([128, L], f32, tag="S0")
            psB = sps.tile([128, L], f32, tag="S1")
            pe_slices = ([(0, 0, lt) for lt in range(3)]
                         + [(bi, cc, lt) for bi in range(BLOC)
                            for cc in range(NCC) for lt in range(2)
                            if (bi, cc) != (0, 0) and (bi, cc, lt) != (1, 3, 1)]
                         + [(1, 3, 1), (1, 3, 3)])
            for sl_i, (bi, cc, lt) in enumerate(pe_slices):
                g = sl_i % 6
                tgt = psA if g % 2 == 0 else psB
                off = (g // 2) * 512
                # lt 1 of the final pair covers only 384 cols; lt 3 is its
                # 128-col completion, so the closing evict+DMA is tiny
                w = 512 if lt < 3 else 128
                if (bi, cc, lt) == (1, 3, 1):
                    w = 384
                for k in range(TOPK):
                    if (k, 0) not in sv_x:
                        pe_snap_lt0(k)
                    if lt > 0 and (k, lt) not in sv_x:
                        pe_snap_lt(k, lt)
                    nc.tensor.matmul(
                        tgt[:, off:off + w],
                        Iw[bi][k],
                        vv[bi][:, cc, bass.ds(sv_x[(k, lt)], w)],
                        start=(k == 0),
                        stop=(k == TOPK - 1),
                        skip_group_check=True,
                    )
                dst0 = 512 * lt if lt < 3 else 896
                ot = obp.tile([128, w], f16, tag="ot")
                nc.scalar.mul(ot, tgt[:, off:off + w], rs[:, bi:bi + 1])
                nc.sync.dma_start(
                    out[bi, 128 * cc:128 * (cc + 1), dst0:dst0 + w],
                    ot,
                )
            for bi in range(BLOC):
                for cc in range(NCC):
                    if bi == 0 and cc == 0:
                        continue
                    acc = sb.tile([128, 512], f16, tag=f"acc{bi}{cc}")
                    tmp = sb.tile([128, 512], f16, tag=f"tmp{bi}{cc}")
                    for k in range(TOPK):
                        dst = acc if k == 0 else tmp
                        nc.vector.tensor_scalar(
                            dst, vv[bi][:, cc, bass.ds(sv_od[k], 512)],
                            wbc[:, 8 * bi + k:8 * bi + k + 1],
                            rs[:, bi:bi + 1],
                            op0=mybir.AluOpType.mult, op1=mybir.AluOpType.mult)
                        if k > 0:
                            nc.vector.tensor_tensor(acc, acc, tmp,
                                                    mybir.AluOpType.add)
                    nc.gpsimd.dma_start(
                        out[bi, 128 * cc:128 * (cc + 1), 1024:L], acc)
    nc.finalize()
    return nc


def _marshal(arr, ncores):
    # [B, L, H, E] fp32 -> per-core contiguous fp16 [BLOC, C, L]
    a = arr.reshape(B, L, C).astype(np.float16)
    a = np.ascontiguousarray(a.transpose(0, 2, 1))  # [B, C, L]
    bloc = B // ncores
    return [a[c * bloc:(c + 1) * bloc] for c in range(ncores)]


def _ensure_axon_hooks_importable():
    # some containers lack antenv.axon_hooks; run_bass_kernel_spmd imports it
    # unconditionally when tracing is requested. A None hook degrades to an
    # untraced run instead of crashing.
    import sys
    import types
    try:
        import antenv.axon_hooks  # noqa: F401
    except ModuleNotFoundError:
        try:
            import antenv
        except ModuleNotFoundError:
            return
        m = types.ModuleType("antenv.axon_hooks")
        m.get_axon_ntff_profile_hook = lambda: None
        sys.modules["antenv.axon_hooks"] = m
        antenv.axon_hooks = m


def kernel(queries, keys, values, attn_mask=None, _trace=False):
    from concourse.bass_utils import run_bass_kernel_spmd

    _ensure_axon_hooks_importable()

    nc = _cache.get("nc")
    if nc is None:
        nc = _build(NCORES)
        _cache["nc"] = nc

    qs = _marshal(np.asarray(queries, np.float32), NCORES)
    ks = _marshal(np.asarray(keys, np.float32), NCORES)
    vs = _marshal(np.asarray(values, np.float32), NCORES)
    in_maps = [{"qT": qs[c], "kT": ks[c], "vT": vs[c]} for c in range(NCORES)]

    res = run_bass_kernel_spmd(nc, in_maps, core_ids=list(range(NCORES)), trace=_trace)
    _cache["last"] = res
    o = np.concatenate([res.results[c]["out"] for c in range(NCORES)], axis=0)
    o = o.transpose(0, 2, 1).astype(np.float32)  # [B, L, C]
    return np.ascontiguousarray(o.reshape(B, L, H, E))


# revision 49
# speedup vs baseline: 1.0020x; 1.0020x over previous
"""AutoCorrelation (FFT-free) kernel for 8 Trainium2 NeuronCores.

Math: the reference computes, per (b, h, e), the circular cross-correlation
corr = irfft(rfft(q) * conj(rfft(k))), then
  mean_corr[b, l] = mean_{h,e} corr          (only this is ever used)
  global_mean[l]  = mean_b mean_corr
  topk lags       = top-7 of global_mean
  weights         = softmax(mean_corr[:, topk])
  out[b,l]        = sum_k w[b,k] * v[b, (l - lag_k) % L]

Identity used: mean_corr[b, l] = (1/HE) * sum_s <q[b,(s+l)%L,:,:], k[b,s,:,:]>.
So instead of FFTs we compute, per batch, the Gram matrix G[s,t] = sum_c
kT[c,s] qT[c,t] on the TensorEngine (fp16 inputs, fp32 PSUM accumulation) and
fold its wrapped diagonals: the fold is fused into the matmul by rotating each
s-chunk's output columns in PSUM, leaving mean_corr[l] = sum_p S[p,(l+p)%L].

That remaining per-partition circular-diagonal fold is done with a skewed
DRAM access pattern instead of a rotate-add tree: S (plus a 128-column
replica) is written to DRAM rows of pitch 1664, then read back through a
pitch-1665 view of the same buffer, which lands S[p, (l+p)%L] at [p, l].
One gpsimd partition_all_reduce then yields the folded vector in a single
instruction. Batch 0's fold runs during batch 1's Gram; after the last
matmul only the COMBINED fold (S0+S1, by linearity = mc0+mc1) is on the
critical path, and batch 1's 7 gathered weights are recovered as
2*gl_local - mc0 without ever folding S1 alone.

Sharding: batch across the 8 cores (2 per core). Only global_mean needs an
AllReduce of a [1,1536] fp32 vector. Top-7 via the DVE max/max_index
instruction (split in halves so it overlaps the fold quarters). The topk
lags become engine registers driving dynamic access-pattern offsets into a
doubled v buffer. The weighted circular gather-sum is lt-split: the PE
covers 17 of the 24 (batch, chunk, 512-col) output slices as w-scaled
identity matmuls over 6 rotating 1-bank PSUM slices (rotating tiles
per-slice, not per-pair, so evictions never stall the next matmul group),
while the DVE covers 7 slices as 4x/2x-mode scale/add tap chains written
out with no PSUM eviction. Weights use an unnormalized softmax
(|mean_corr| <= ~8 so exp(x/C) stays in fp16/fp32 range) with 1/sum folded
into the eviction scale / tap scale; batch 0's exp->broadcast->Iw chain is
emitted first so it alone gates the first output matmuls. Loads are
ordered k0/q0 (chunk 0 in column halves) -> k1/q1 -> v on one queue: the
cost model's HWDGE and DMA units are single-slot, so issue order is
transfer order, and the Gram's cc-outer loop starts ~3us in on chunk 0
alone.

fp16 is safe here: top-7 global_mean gap is 1.5e-3 while fp16-input error is
<5e-4 (validated against the fp32 FFT reference), and the output tolerance is
2e-2 vs our ~7e-4.
"""

import numpy as np

B, L, H, E = 16, 1536, 8, 64
C = H * E             # 512 channels = H*E
NCORES = 8
BLOC = B // NCORES    # batches per core
NCC = C // 128        # channel chunks of 128
TOPK = 7              # int(1 * log(1536)) == 7
NJ = L // 128         # s-chunks
NLT = L // 512        # output l-tiles
PITCH = 1664          # DRAM row pitch for the skewed diagonal fold

_cache = {}
DEBUG_BUILD = False


def _build(num_cores: int):
    import concourse.bass as bass
    import concourse.bacc as bacc
    import concourse.mybir as mybir
    import concourse.tile as tile
    from concourse import bass_isa

    f16 = mybir.dt.float16
    f32 = mybir.dt.float32
    u32 = mybir.dt.uint32
    PE = mybir.EngineType.PE
    ACT = mybir.EngineType.Activation
    DVE = mybir.EngineType.DVE

    nc = bacc.Bacc(None)
    qT = nc.dram_tensor("qT", [BLOC, C, L], f16, kind="ExternalInput")
    kT = nc.dram_tensor("kT", [BLOC, C, L], f16, kind="ExternalInput")
    vT = nc.dram_tensor("vT", [BLOC, C, L], f16, kind="ExternalInput")
    out = nc.dram_tensor("out", [BLOC, C, L], f16, kind="ExternalOutput")
    ident_d = nc.inline_tensor(np.eye(128, dtype=np.float16), "identc")
    if DEBUG_BUILD:
        dbg_mc0 = nc.dram_tensor("dbg_mc0", [1, L], f32, kind="ExternalOutput")
        dbg_gl = nc.dram_tensor("dbg_gl", [1, L], f32, kind="ExternalOutput")
        dbg_idx = nc.dram_tensor("dbg_idx", [1, 8], u32, kind="ExternalOutput")
        dbg_wq = nc.dram_tensor("dbg_wq", [1, 16], f32, kind="ExternalOutput")
        dbg_ex = nc.dram_tensor("dbg_ex", [1, 16], f32, kind="ExternalOutput")
        dbg_rs = nc.dram_tensor("dbg_rs", [1, 2], f32, kind="ExternalOutput")
        dbg_rmax = nc.dram_tensor("dbg_rmax", [1, 1], f32, kind="ExternalOutput")
        dbg_wqs = nc.dram_tensor("dbg_wqs", [1, 16], f32, kind="ExternalOutput")
        dbg_wbc = nc.dram_tensor("dbg_wbc", [2, 16], f32, kind="ExternalOutput")

    def skew_view(d):
        # pitch-(PITCH+1) view of a [129, PITCH] dram tile: element [p, l]
        # aliases row p, column p+l of the pitch-PITCH writes, i.e. the
        # circular diagonal once columns [0,128) are replicated at [L,PITCH).
        flat = d[:].rearrange("a b -> (a b)")
        return flat[0:128 * (PITCH + 1)].rearrange("(a b) -> a b", b=PITCH + 1)

    with tile.TileContext(nc) as tc:
        with (
            tc.tile_pool(name="sb", bufs=1) as sb,
            tc.tile_pool(name="sps", bufs=1, space="PSUM") as sps,
            tc.tile_pool(name="obp", bufs=3) as obp,
            tc.tile_pool(name="dram", bufs=1, space="DRAM") as dram,
        ):
            ident = sb.tile([128, 128], f16, tag="ident")
            nc.sync.dma_start(ident, ident_d[:])

            # ---- loads: k0/q0 in interleaved halves so the Gram starts
            # ~6us in; k1/q1/v behind them on the (serial) HWDGE+DMA units.
            kq = {}
            for (nm, t) in (("k", kT), ("q", qT)):
                for bi in range(BLOC):
                    kqt = sb.tile([128, NCC, L], f16, tag=f"{nm}{bi}")
                    kq[(nm, bi)] = kqt
            for (nm, h) in (("k", 0), ("q", 0), ("q", 1), ("k", 1)):
                src = (kT if nm == "k" else qT)[0].rearrange(
                    "(cc p) l -> p cc l", p=128)
                nc.sync.dma_start(
                    kq[(nm, 0)][:, 0:1, 768 * h:768 * (h + 1)],
                    src[:, 0:1, 768 * h:768 * (h + 1)])
            for (a, b) in ((1, 2), (2, 4)):
                for nm in ("k", "q"):
                    src = (kT if nm == "k" else qT)[0].rearrange(
                        "(cc p) l -> p cc l", p=128)
                    nc.sync.dma_start(kq[(nm, 0)][:, a:b, :], src[:, a:b, :])
            for nm in ("k", "q"):
                src = (kT if nm == "k" else qT)[1].rearrange(
                    "(cc p) l -> p cc l", p=128)
                nc.sync.dma_start(kq[(nm, 1)][:], src)
            # v loads go last on the same queue: the single HWDGE/DMA unit
            # serves transfers in issue order, and v isn't needed until the
            # output phase.
            vv = []
            for bi in range(BLOC):
                t = sb.tile([128, NCC, 2 * L], f16, tag=f"vv{bi}")
                nc.sync.dma_start(
                    t[:, :, 0:L], vT[bi].rearrange("(cc p) l -> p cc l", p=128))
                vv.append(t)
            # duplicate halves on the VectorEngine instead of re-reading HBM
            for bi in range(BLOC):
                nc.vector.tensor_copy(vv[bi][:, :, L:2 * L], vv[bi][:, :, 0:L])

            # prime the ACT exp table off the critical path
            dume = sb.tile([1, 1], f32, tag="dume")
            nc.scalar.activation(dume, ident[0:1, 0:1],
                                 mybir.ActivationFunctionType.Exp,
                                 bias=0.0, scale=1.0)
            # init wq sentinel so the max over all 16 columns is safe
            wq = sb.tile([1, 16], f32, tag="wq")
            nc.vector.memset(wq, -1e30)

            # ---- Gram with rotated PSUM accumulation ----
            # cc-pair outer so batch 0's Gram can start once the first-half
            # (chunks 0-1) k/q DMAs land, streaming the rest underneath.
            def gram(bi):
                S = sps.tile([128, L], f32, tag=f"S{bi}")
                for cc in range(NCC):
                    for u in range(NJ):
                        r = (L - 128 * u) % L
                        segs = []
                        t0 = 0
                        while t0 < L:
                            y0 = (t0 + r) % L
                            seg = min(512 - (y0 % 512), L - t0, L - y0)
                            segs.append((t0, y0, seg))
                            t0 += seg
                        for (ts_, ys_, seg) in sorted(segs, key=lambda s: s[1]):
                            nc.tensor.matmul(
                                S[:, ys_:ys_ + seg],
                                kq[("k", bi)][:, cc, 128 * u:128 * (u + 1)],
                                kq[("q", bi)][:, cc, ts_:ts_ + seg],
                                start=(cc == 0 and u == 0),
                                stop=(cc == NCC - 1 and u == NJ - 1),
                                skip_group_check=True,
                            )
                return S

            S0 = gram(0)

            # ---- batch-0 diagonal fold (runs during batch-1 Gram) ----
            E0 = sb.tile([128, L], f32, tag="E0")
            nc.vector.tensor_copy(E0, S0)
            D0 = dram.tile([129, PITCH], f32)
            nc.sync.dma_start(D0[0:128, 0:L], E0)
            nc.scalar.dma_start(D0[0:128, L:PITCH], E0[:, 0:128])
            R0 = sb.tile([128, L], f32, tag="R0")
            nc.sync.dma_start(R0, skew_view(D0)[:, 0:L])
            mc0 = sb.tile([128, L], f32, tag="mc0")
            nc.gpsimd.partition_all_reduce(mc0, R0, channels=128,
                                           reduce_op=bass_isa.ReduceOp.add)
            # precompute exp(mc0/C) off the critical path: the weight
            # gathers then read ready-made exponentials
            emc0 = sb.tile([1, L], f32, tag="emc0")
            nc.scalar.activation(emc0, mc0[0:1, :],
                                 mybir.ActivationFunctionType.Exp,
                                 bias=0.0, scale=1.0 / C)

            S1 = gram(1)

            # ---- combined fold: gl_local = fold(S0+S1) = mc0 + mc1 ----
            # quarter-pipelined: combine -> write -> skewed read -> PAR all
            # stream through the (serial) DMA unit in 384-column quarters.
            # fp16 round trip: S entries are ~N(0, 22^2) so fp16 keeps the
            # fold's absolute error ~0.7 vs a 209 top-7/8 global_mean gap,
            # and halves the critical-path DMA transfer time. PAR upcasts to
            # fp32 internally and glp stays fp32.
            G = sb.tile([128, L], f16, tag="G")
            QL = L // 4
            Dg = dram.tile([129, PITCH], f16)
            Rg = sb.tile([128, L], f16, tag="Rg")
            glp = sb.tile([128, L], f32, tag="glp")
            sv = skew_view(Dg)
            # E0 is S0 already evicted to SBUF: keeps this add to a single
            # PSUM operand (two PSUM reads in one op fail the BIR verifier)
            for j in range(4):
                a, b = QL * j, QL * (j + 1)
                nc.vector.tensor_tensor(G[:, a:b], E0[:, a:b], S1[:, a:b],
                                        mybir.AluOpType.add)
                nc.sync.dma_start(Dg[0:128, a:b], G[:, a:b])
                if j == 0:
                    nc.scalar.dma_start(Dg[0:128, L:PITCH], G[:, 0:128])
            # read quarter j touches row columns [QL*j, QL*j+QL+127): needs
            # write quarters j..j+1 (and the replica for the last one).
            egl = sb.tile([1, L], f32, tag="egl")
            for j in range(4):
                a, b = QL * j, QL * (j + 1)
                nc.sync.dma_start(Rg[:, a:b], sv[:, a:b])
                nc.gpsimd.partition_all_reduce(glp[:, a:b], Rg[:, a:b],
                                               channels=128,
                                               reduce_op=bass_isa.ReduceOp.add)
                nc.scalar.activation(egl[0:1, a:b], glp[0:1, a:b],
                                     mybir.ActivationFunctionType.Exp,
                                     bias=0.0, scale=1.0 / C)

            # ---- cross-core AllReduce of the local global-mean ----
            if num_cores > 1:
                cc_in = dram.tile([1, L], f32)
                cc_out = dram.tile([1, L], f32)
                nc.sync.dma_start(cc_in, glp[0:1, :])
                nc.gpsimd.collective_compute(
                    "AllReduce",
                    mybir.AluOpType.add,
                    replica_groups=[list(range(num_cores))],
                    ins=[cc_in.opt()],
                    outs=[cc_out.opt()],
                )
                gm = sb.tile([1, L], f32, tag="gm")
                nc.sync.dma_start(gm, cc_out)
            else:
                gm = glp[0:1, :]

            # ---- top-7 lags (top-8 instruction, first 7 used) ----
            # halves overlap the PAR quarters; merged top-8 feeds the
            # full-width index search.
            vals = sb.tile([1, 8], f32, tag="vals")
            idxs = sb.tile([1, 8], u32, tag="idxs")
            v16 = sb.tile([1, 16], f32, tag="v16")
            nc.vector.max(v16[0:1, 0:8], gm[0:1, 0:L // 2])
            nc.vector.max(v16[0:1, 8:16], gm[0:1, L // 2:L])
            nc.vector.max(vals, v16)
            nc.vector.max_index(idxs, vals, gm)

            # ---- index registers ----
            act_eng = nc.engines[ACT]
            dve_eng = nc.engines[DVE]
            pe_eng = nc.engines[PE]
            sv_x = {}    # PE: (L - idx) + 512*lt for the dynamic matmuls
            sv_gd = []   # DVE: idx for mc0 gathers
            sv_od = []   # DVE: L - idx for the cc=3 tap windows
            sv_ga = []   # ACT: idx for mc0 taps 4-6
            sv_ga2 = []  # ACT: idx for gl gathers
            # PE index registers are emitted lazily, interleaved with the
            # first output slices, so the ~100ns/op SEQ register setup hides
            # behind matmul engine time instead of gating the first Ldweights
            pe_ro = {}

            def pe_snap_lt0(k):
                rp = pe_eng.alloc_register(f"ip{k}")
                pe_eng.reg_load(rp, idxs[0:1, k:k + 1])
                ro = pe_eng.alloc_register(f"io{k}")
                pe_eng.reg_alu(ro, L, rp, mybir.AluOpType.subtract)
                pe_eng.free_register(rp)
                pe_ro[k] = ro
                sv_x[(k, 0)] = pe_eng.snap(ro, donate=False,
                                           min_val=1, max_val=L)

            def pe_snap_lt(k, lt):
                # lt 3 = the final 128-col mini-slice at offset 512+384
                base = 512 * lt if lt < 3 else 896
                rx = pe_eng.alloc_register(f"ix{k}_{lt}")
                pe_eng.reg_alu(rx, pe_ro[k], base, mybir.AluOpType.add)
                if lt == 3:
                    pe_eng.free_register(pe_ro[k])
                sv_x[(k, lt)] = pe_eng.snap(rx, donate=True,
                                            min_val=base + 1,
                                            max_val=L + base)

            for k in range(TOPK):
                ra = act_eng.alloc_register(f"ia{k}")
                act_eng.reg_load(ra, idxs[0:1, k:k + 1])
                sv_ga.append(act_eng.snap(ra, donate=True,
                                          min_val=0, max_val=L - 1))
            dve_regs = []
            for k in range(TOPK):
                rd = dve_eng.alloc_register(f"id{k}")
                dve_eng.reg_load(rd, idxs[0:1, k:k + 1])
                dve_regs.append(rd)
                sv_gd.append(dve_eng.snap(rd, donate=False,
                                          min_val=0, max_val=L - 1))

            # ---- weights: wq[0,0:7]=mc0 taps, wq[0,8:15]=gl-mc0 taps ----
            # mc0 gathers split DVE/ACT so exp-b0 fires ~0.7us after topk;
            # the gl gathers (only needed for b1) follow on ACT afterwards.
            for k in range(4):
                nc.vector.tensor_copy(wq[0:1, k:k + 1],
                                      emc0[0:1, bass.ds(sv_gd[k], 1)])
            for k in range(4, TOPK):
                nc.scalar.copy(wq[0:1, k:k + 1],
                               emc0[0:1, bass.ds(sv_ga[k], 1)])
            for k in range(TOPK):
                rdo = dve_eng.alloc_register(f"do{k}")
                dve_eng.reg_alu(rdo, L + 1024, dve_regs[k],
                                mybir.AluOpType.subtract)
                dve_eng.free_register(dve_regs[k])
                sv_od.append(dve_eng.snap(rdo, donate=True,
                                          min_val=1025, max_val=L + 1024))
            # b0's weight pipeline runs first and alone gates the first PE
            # slices; b1 (= gl - mc0 by linearity, then exp/bcast/Iw) follows
            # in its shadow. Softmax is unnormalized: |mean_corr| <~ 8 so
            # exp(x/C) <~ e^8 fits fp16/fp32; 1/sum folds into output scaling.
            ex = sb.tile([1, 16], f32, tag="ex")
            wbc = sb.tile([128, 16], f32, tag="wbc")
            rs = sb.tile([128, 2], f32, tag="rs")
            sm = sb.tile([128, 2], f32, tag="sm")
            Iw = [[None] * TOPK for _ in range(BLOC)]
            nc.gpsimd.partition_broadcast(wbc[:, 0:8], wq[0:1, 0:8],
                                          channels=128)
            for k in range(TOPK):
                ra2 = act_eng.alloc_register(f"ib{k}")
                act_eng.reg_load(ra2, idxs[0:1, k:k + 1])
                sv_ga2.append(act_eng.snap(ra2, donate=True,
                                           min_val=0, max_val=L - 1))
                nc.scalar.copy(wq[0:1, 8 + k:9 + k],
                               egl[0:1, bass.ds(sv_ga2[k], 1)])
            for k in range(TOPK):
                t = sb.tile([128, 128], f16, tag=f"iw0{k}")
                nc.vector.tensor_scalar_mul(t, ident, wbc[:, k:k + 1])
                Iw[0][k] = t
            # gl is the SUM over this core's two batches, so in exp space
            # w1_k = exp(mc1_k/C) = egl_k / emc0_k
            rex = sb.tile([1, 8], f32, tag="rex")
            nc.vector.reciprocal(rex[0:1, 0:7], wq[0:1, 0:7])
            nc.vector.tensor_tensor(wq[0:1, 8:15], wq[0:1, 8:15],
                                    rex[0:1, 0:7], mybir.AluOpType.mult)
            nc.gpsimd.partition_broadcast(wbc[:, 8:16], wq[0:1, 8:16],
                                          channels=128)
            for k in range(TOPK):
                t = sb.tile([128, 128], f16, tag=f"iw1{k}")
                nc.gpsimd.tensor_scalar_mul(t, ident, wbc[:, 8 + k:9 + k])
                Iw[1][k] = t
            nc.vector.tensor_reduce(sm[:, 0:1], wbc[:, 0:TOPK],
                                    mybir.AxisListType.X, mybir.AluOpType.add)
            nc.vector.tensor_reduce(sm[:, 1:2], wbc[:, 8:8 + TOPK],
                                    mybir.AxisListType.X, mybir.AluOpType.add)
            nc.vector.reciprocal(rs, sm)
            # window offsets for the DVE slices: (L + 512*lt) - idx for
            # lt in {1,2}, off the critical path (only needed once the DVE
            # tap chains start)

            if DEBUG_BUILD:
                nc.sync.dma_start(dbg_mc0[:], mc0[0:1, :])
                nc.sync.dma_start(dbg_gl[:], glp[0:1, :])
                nc.sync.dma_start(dbg_idx[:], idxs)
                nc.sync.dma_start(dbg_wq[:], wq)
                nc.sync.dma_start(dbg_ex[:], ex)
                nc.sync.dma_start(dbg_rs[:], rs[0:1, :])
                nc.sync.dma_start(dbg_wbc[:], wbc[0:2, :])

            # ---- weighted circular gather-sum ----
            # chunks 0-2: w-scaled identity matmuls, dynamic rhs offsets,
            # PSUM accumulation; eviction applies the 1/sum normalization.
            # chunk 3: DVE scale/add chain on full-L windows (runs in the
            # shadow of the PE matmuls).
            # lt-split output: PE computes lt 0-1 of every (batch, chunk)
            # pair as w-scaled identity matmuls (6 rotating 512-col PSUM
            # slices so evictions never stall the next slice's matmuls); the
            # DVE computes every lt=2 slice as a scale/add tap chain and DMAs
            # it out directly, so both engines finish together and the kernel
            # ends on an eviction-free DMA.
            psA = sps.tile([128, L], f32, tag="S0")
            psB = sps.tile([128, L], f32, tag="S1")
            pe_slices = ([(0, 0, lt) for lt in range(3)]
                         + [(bi, cc, lt) for bi in range(BLOC)
                            for cc in range(NCC) for lt in range(2)
                            if (bi, cc) != (0, 0) and (bi, cc, lt) != (1, 3, 1)]
                         + [(1, 3, 1), (1, 3, 3)])
            for sl_i, (bi, cc, lt) in enumerate(pe_slices):
                g = sl_i % 6
                tgt = psA if g % 2 == 0 else psB
                off = (g // 2) * 512
                # lt 1 of the final pair covers only 384 cols; lt 3 is its
                # 128-col completion, so the closing evict+DMA is tiny
                w = 512 if lt < 3 else 128
                if (bi, cc, lt) == (1, 3, 1):
                    w = 384
                for k in range(TOPK):
                    if (k, 0) not in sv_x:
                        pe_snap_lt0(k)
                    if lt > 0 and (k, lt) not in sv_x:
                        pe_snap_lt(k, lt)
                    nc.tensor.matmul(
                        tgt[:, off:off + w],
                        Iw[bi][k],
                        vv[bi][:, cc, bass.ds(sv_x[(k, lt)], w)],
                        start=(k == 0),
                        stop=(k == TOPK - 1),
                        skip_group_check=True,
                    )
                dst0 = 512 * lt if lt < 3 else 896
                ot = obp.tile([128, w], f16, tag="ot")
                nc.scalar.mul(ot, tgt[:, off:off + w], rs[:, bi:bi + 1])
                nc.sync.dma_start(
                    out[bi, 128 * cc:128 * (cc + 1), dst0:dst0 + w],
                    ot,
                )
            for bi in range(BLOC):
                for cc in range(NCC):
                    if bi == 0 and cc == 0:
                        continue
                    acc = sb.tile([128, 512], f16, tag=f"acc{bi}{cc}")
                    tmp = sb.tile([128, 512], f16, tag=f"tmp{bi}{cc}")
                    for k in range(TOPK):
                        dst = acc if k == 0 else tmp
                        nc.vector.tensor_scalar(
                            dst, vv[bi][:, cc, bass.ds(sv_od[k], 512)],
                            wbc[:, 8 * bi + k:8 * bi + k + 1],
                            rs[:, bi:bi + 1],
                            op0=mybir.AluOpType.mult, op1=mybir.AluOpType.mult)
                        if k > 0:
                            nc.vector.tensor_tensor(acc, acc, tmp,
                                                    mybir.AluOpType.add)
                    nc.gpsimd.dma_start(
                        out[bi, 128 * cc:128 * (cc + 1), 1024:L], acc)
    nc.finalize()
    return nc


def _marshal(arr, ncores):
    # [B, L, H, E] fp32 -> per-core contiguous fp16 [BLOC, C, L]
    a = arr.reshape(B, L, C).astype(np.float16)
    a = np.ascontiguousarray(a.transpose(0, 2, 1))  # [B, C, L]
    bloc = B // ncores
    return [a[c * bloc:(c + 1) * bloc] for c in range(ncores)]


def _ensure_axon_hooks_importable():
    # some containers lack antenv.axon_hooks; run_bass_kernel_spmd imports it
    # unconditionally when tracing is requested. A None hook degrades to an
    # untraced run instead of crashing.
    import sys
    import types
    try:
        import antenv.axon_hooks  # noqa: F401
    except ModuleNotFoundError:
        try:
            import antenv
        except ModuleNotFoundError:
            return
        m = types.ModuleType("antenv.axon_hooks")
        m.get_axon_ntff_profile_hook = lambda: None
        sys.modules["antenv.axon_hooks"] = m
        antenv.axon_hooks = m


def kernel(queries, keys, values, attn_mask=None, _trace=False):
    from concourse.bass_utils import run_bass_kernel_spmd

    _ensure_axon_hooks_importable()

    nc = _cache.get("nc")
    if nc is None:
        nc = _build(NCORES)
        _cache["nc"] = nc

    qs = _marshal(np.asarray(queries, np.float32), NCORES)
    ks = _marshal(np.asarray(keys, np.float32), NCORES)
    vs = _marshal(np.asarray(values, np.float32), NCORES)
    in_maps = [{"qT": qs[c], "kT": ks[c], "vT": vs[c]} for c in range(NCORES)]

    res = run_bass_kernel_spmd(nc, in_maps, core_ids=list(range(NCORES)), trace=_trace)
    _cache["last"] = res
    o = np.concatenate([res.results[c]["out"] for c in range(NCORES)], axis=0)
    o = o.transpose(0, 2, 1).astype(np.float32)  # [B, L, C]
    return np.ascontiguousarray(o.reshape(B, L, H, E))
